# revision 1
# baseline (speedup 1.0000x reference)
"""Trainium2 Bass kernel for nn_MBSFeedForward (moe_routing).

Reference semantics (per token t with class c = type_seq[t]):
  c == 0:  out = LN_out(x_t)
  c >= 1:  e = c-1 (expert)
           u = GELU(x_t @ W1_e + b1_e) @ W2_e + b2_e
           v = LN_e(u + x_t)          (per-expert ln_g/ln_b)
           out = LN_out(v + x_t)      (out_g/out_b)

Sharding strategy (routing done on host as part of sharding):
  - 4 experts x 2 cores each; each core processes half of its expert's
    tokens through the full FFN chain.
  - class-0 tokens are split evenly over all 8 cores and only go through
    the outer LayerNorm.
  - No collectives needed; host scatters per-core outputs back.

Device kernel (per core, SPMD — same NEFF, different data):
  GEMM1 computes GELU(W1^T x + b1) in [F, tok] layout (F on partitions)
  which feeds GEMM2 as the stationary operand producing U in [tok, H]
  layout (tokens on partitions) — no transposes anywhere; the host
  supplies x both natural and pre-transposed. b2 is accumulated into
  PSUM with a rank-1 (K=1) ones x b2 matmul. LayerNorms use
  bn_stats/bn_aggr with tokens on partitions.
"""

import math
from contextlib import ExitStack

import numpy as np

import concourse.bass as bass
import concourse.tile as tile
from concourse import bacc
from concourse import mybir
from concourse.bass_utils import run_bass_kernel_spmd

F32 = mybir.dt.float32
F32R = mybir.dt.float32r

P = 128
H = 768
F = 3072
KH = H // P    # 6
KF = F // P    # 24
TCH = 256      # tokens per chunk (GEMM1 moving free dim; >=256 keeps f32r at full rate)
TPT = TCH // P # token tiles per chunk
NCORES = 8
EPS = 1e-12

# swappable for CoreSim validation (Gelu not implemented in the interpreter)
ACT_FUNC = mybir.ActivationFunctionType.Gelu


def round_f32r(a: np.ndarray) -> np.ndarray:
    """Round fp32 to fp32r (e8m11: low 12 mantissa bits dropped, RNE) —
    matches walrus fp32_to_fp32r; required for tensors fed to f32r matmuls."""
    u = np.ascontiguousarray(a, dtype=np.float32).view(np.uint32)
    r = (u + np.uint32(0x7FF) + ((u >> np.uint32(12)) & np.uint32(1))) & np.uint32(0xFFFFF000)
    return r.view(np.float32)


def build_nc(cap: int, cap0: int, repeat: int = 1, *, affine_pool: bool = False,
             ps1_bufs: int = 4, ha_bufs: int = 10, psu_bufs: int = 2,
             work_bufs: int = 3, xin_bufs: int = 2, xres_bufs: int = 2,
             stat_bufs: int = 4) -> bass.Bass:
    """Build the per-core Bass module for `cap` routed tokens and `cap0`
    LN-only tokens (both multiples of TCH / P respectively).

    repeat > 1 re-runs the whole computation (idempotent) — used only for
    slope-based timing of the steady-state kernel body."""
    assert cap % TCH == 0 and cap0 % P == 0
    nc = bacc.Bacc()

    d_xr = nc.dram_tensor("xr", [cap, H], F32, kind="ExternalInput")
    d_xrT = nc.dram_tensor("xrT", [H, cap], F32R, kind="ExternalInput")
    d_x0 = nc.dram_tensor("x0", [cap0, H], F32, kind="ExternalInput")
    d_w1 = nc.dram_tensor("w1", [H, F], F32R, kind="ExternalInput")
    d_b1 = nc.dram_tensor("b1", [F], F32, kind="ExternalInput")
    d_w2 = nc.dram_tensor("w2", [F, H], F32R, kind="ExternalInput")
    d_b2 = nc.dram_tensor("b2", [H], F32R, kind="ExternalInput")
    d_lng = nc.dram_tensor("lng", [H], F32, kind="ExternalInput")
    d_lnb = nc.dram_tensor("lnb", [H], F32, kind="ExternalInput")
    d_outg = nc.dram_tensor("outg", [H], F32, kind="ExternalInput")
    d_outb = nc.dram_tensor("outb", [H], F32, kind="ExternalInput")
    d_yr = nc.dram_tensor("yr", [cap, H], F32, kind="ExternalOutput")
    d_y0 = nc.dram_tensor("y0", [cap0, H], F32, kind="ExternalOutput")

    with tile.TileContext(nc) as tc, ExitStack() as ctx:
        singles = ctx.enter_context(tc.tile_pool(name="singles", bufs=1))
        xin = ctx.enter_context(tc.tile_pool(name="xin", bufs=xin_bufs))
        xres = ctx.enter_context(tc.tile_pool(name="xres", bufs=xres_bufs))
        hpool = ctx.enter_context(tc.tile_pool(name="hact", bufs=ha_bufs))
        work = ctx.enter_context(tc.tile_pool(name="work", bufs=work_bufs))
        stat = ctx.enter_context(tc.tile_pool(name="stat", bufs=stat_bufs))
        ps1p = ctx.enter_context(tc.tile_pool(name="ps1", bufs=ps1_bufs, space="PSUM"))
        psUp = ctx.enter_context(tc.tile_pool(name="psU", bufs=psu_bufs, space="PSUM"))

        # --- resident weights / constants ---
        # weights stream on the ACT queue, interleaved in the order chunk 0
        # consumes them (w1 F-group fg feeds GEMM1 kf in [4fg, 4fg+4); w2
        # ko-slice kf feeds GEMM2 kf) so compute starts ~immediately and
        # chunk 0 runs at DMA pace instead of stalling on a bulk load.
        w1sb = singles.tile([P, KH, F], F32R)
        w1_v = d_w1[:].rearrange("(ko ki) f -> ki ko f", ki=P)
        w2sb = singles.tile([P, KF, H], F32R)
        w2_v = d_w2[:].rearrange("(ko ki) h -> ki ko h", ki=P)
        for fg in range(F // 512):
            nc.scalar.dma_start(
                w1sb[:, :, fg * 512:(fg + 1) * 512],
                w1_v[:, :, fg * 512:(fg + 1) * 512],
            )
            for ko in range(4 * fg, 4 * fg + 4):
                nc.scalar.dma_start(w2sb[:, ko], w2_v[:, ko])
        b1sb = singles.tile([P, KF], F32)
        nc.sync.dma_start(b1sb, d_b1[:].rearrange("(o p) -> p o", p=P))

        def bc_tile(d, nm):
            t = singles.tile([P, H], F32, tag=nm, name=nm)
            nc.gpsimd.dma_start(t, d[:][None, :].to_broadcast([P, H]))
            return t

        lngbc = bc_tile(d_lng, "lngbc")
        lnbbc = bc_tile(d_lnb, "lnbbc")
        outgbc = bc_tile(d_outg, "outgbc")
        outbbc = bc_tile(d_outb, "outbbc")

        ones_f32 = singles.tile([1, P], F32)
        nc.vector.memset(ones_f32, 1.0)
        ones_t = singles.tile([1, P], F32R)
        nc.vector.tensor_copy(ones_t, ones_f32)
        b2row = singles.tile([1, H], F32R)
        nc.sync.dma_start(b2row, d_b2[:][None, :])
        eps_t = singles.tile([P, 1], F32)
        nc.vector.memset(eps_t, EPS)

        SG = H // 256  # bn_stats subgroups of 256

        def layernorm_inplace(xt, gbc, bbc):
            st = stat.tile([P, SG, 6], F32, tag="st")
            for sg in range(SG):
                nc.vector.bn_stats(st[:, sg, :], xt[:, sg * 256:(sg + 1) * 256])
            mv = stat.tile([P, 2], F32, tag="mv")
            nc.vector.bn_aggr(mv, st)
            std = stat.tile([P, 1], F32, tag="std")
            nc.scalar.activation(
                std, mv[:, 1:2], mybir.ActivationFunctionType.Sqrt,
                bias=eps_t, scale=1.0,
            )
            rs = stat.tile([P, 1], F32, tag="rs")
            nc.vector.reciprocal(rs, std)
            nc.vector.tensor_scalar(
                xt, xt, scalar1=mv[:, 0:1], scalar2=rs,
                op0=mybir.AluOpType.subtract, op1=mybir.AluOpType.mult,
            )
            eng = nc.gpsimd if affine_pool else nc.vector
            eng.tensor_mul(xt, xt, gbc)
            eng.tensor_add(xt, xt, bbc)

        xrT_v = d_xrT[:].rearrange("(ko ki) t -> ki ko t", ki=P)

        # --- class-0 tokens: outer LayerNorm only (runs during weight load) ---
        for t in range(repeat * (cap0 // P)):
            t = t % (cap0 // P)
            x0t = work.tile([P, H], F32, tag="r")
            nc.sync.dma_start(x0t, d_x0[t * P:(t + 1) * P, :])
            layernorm_inplace(x0t, outgbc, outbbc)
            nc.sync.dma_start(d_y0[t * P:(t + 1) * P, :], x0t)

        # --- routed tokens: full FFN chain ---
        for c in range(repeat * (cap // TCH)):
            c = c % (cap // TCH)
            xrTt = xin.tile([P, KH, TCH], F32R)
            nc.gpsimd.dma_start(xrTt, xrT_v[:, :, c * TCH:(c + 1) * TCH])
            psUs = [
                psUp.tile([P, H], F32, tag="psU", name=f"psU_{m}")
                for m in range(TPT)
            ]
            for kf in range(KF):
                ps1 = ps1p.tile([P, TCH], F32, tag="ps1")
                for kh in range(KH):
                    nc.tensor.matmul(
                        ps1,
                        w1sb[:, kh, kf * P:(kf + 1) * P],
                        xrTt[:, kh, :],
                        start=(kh == 0), stop=(kh == KH - 1),
                    )
                # gelu(ps1 + b1[f-tile]) — bias is per-partition (F on partitions)
                ha = hpool.tile([P, TCH], F32R)
                nc.scalar.activation(
                    ha, ps1, ACT_FUNC,
                    bias=b1sb[:, kf:kf + 1], scale=1.0,
                )
                for m in range(TPT):
                    for h0, h1 in ((0, 512), (512, H)):
                        nc.tensor.matmul(
                            psUs[m][:, h0:h1],
                            ha[:, m * P:(m + 1) * P],
                            w2sb[:, kf, h0:h1],
                            start=(kf == 0), stop=False,
                        )
            # U += ones ⊗ b2 (rank-1 bias add directly in PSUM)
            for m in range(TPT):
                for h0, h1 in ((0, 512), (512, H)):
                    nc.tensor.matmul(
                        psUs[m][:, h0:h1], ones_t, b2row[:, h0:h1],
                        start=False, stop=True,
                    )
            for m in range(TPT):
                tok0 = c * TCH + m * P
                xrt = xres.tile([P, H], F32)
                nc.sync.dma_start(xrt, d_xr[tok0:tok0 + P, :])
                r = work.tile([P, H], F32, tag="r")
                nc.vector.tensor_add(r, psUs[m], xrt)
                layernorm_inplace(r, lngbc, lnbbc)
                nc.vector.tensor_add(r, r, xrt)
                layernorm_inplace(r, outgbc, outbbc)
                nc.sync.dma_start(d_yr[tok0:tok0 + P, :], r)

    nc.finalize()
    return nc


_NC_CACHE: dict[tuple[int, int], bass.Bass] = {}


def get_nc(cap: int, cap0: int, repeat: int = 1) -> bass.Bass:
    key = (cap, cap0, repeat)
    if key not in _NC_CACHE:
        _NC_CACHE[key] = build_nc(cap, cap0, repeat)
    return _NC_CACHE[key]


def _round_up(n: int, m: int) -> int:
    return max(m, ((n + m - 1) // m) * m)


def shard_inputs(input_tensor, type_seq, W1, b1, W2, b2, ln_g, ln_b, out_g, out_b):
    """Host-side routing/sharding. Returns (in_maps, core_tokens, zero_splits,
    cap, cap0)."""
    B, L, _H = input_tensor.shape
    assert _H == H, f"kernel hardcodes d_model={H}, got {_H}"
    x = np.ascontiguousarray(np.asarray(input_tensor, dtype=np.float32)).reshape(B * L, H)
    ts_flat = np.asarray(type_seq).reshape(-1).astype(np.int64)
    NB = W1.shape[0]
    per_expert = max(1, NCORES // NB)

    core_tokens = []
    core_expert = []
    for e in range(NB):
        toks = np.nonzero(ts_flat == e + 1)[0]
        for s in np.array_split(toks, per_expert):
            core_tokens.append(s)
            core_expert.append(e)
    while len(core_tokens) < NCORES:  # NB not dividing NCORES: idle cores
        core_tokens.append(np.zeros(0, dtype=np.int64))
        core_expert.append(0)
    zero_splits = np.array_split(np.nonzero(ts_flat == 0)[0], NCORES)

    cap = _round_up(max(len(t) for t in core_tokens), TCH)
    cap0 = _round_up(max(len(z) for z in zero_splits), P)

    def f32c(a):
        return np.ascontiguousarray(np.asarray(a, dtype=np.float32))

    in_maps = []
    for c in range(NCORES):
        toks = core_tokens[c]
        e = core_expert[c]
        z = zero_splits[c]
        xr = np.zeros((cap, H), np.float32)
        xr[: len(toks)] = x[toks]
        x0 = np.zeros((cap0, H), np.float32)
        x0[: len(z)] = x[z]
        in_maps.append({
            "xr": xr,
            "xrT": round_f32r(np.ascontiguousarray(xr.T)),
            "x0": x0,
            "w1": round_f32r(W1[e]),
            "b1": f32c(b1[e]),
            "w2": round_f32r(W2[e]),
            "b2": round_f32r(b2[e]),
            "lng": f32c(ln_g[e]),
            "lnb": f32c(ln_b[e]),
            "outg": f32c(out_g),
            "outb": f32c(out_b),
        })
    return in_maps, core_tokens, zero_splits, cap, cap0


def unshard_output(results, core_tokens, zero_splits, shape, dtype):
    B, L, _H = shape
    out = np.empty((B * L, H), np.float32)
    for c in range(NCORES):
        toks = core_tokens[c]
        z = zero_splits[c]
        if len(toks):
            out[toks] = results[c]["yr"][: len(toks)]
        if len(z):
            out[z] = results[c]["y0"][: len(z)]
    return out.reshape(B, L, H).astype(dtype, copy=False)


def kernel(input_tensor, type_seq, W1, b1, W2, b2, ln_g, ln_b, out_g, out_b):
    in_maps, core_tokens, zero_splits, cap, cap0 = shard_inputs(
        input_tensor, type_seq, W1, b1, W2, b2, ln_g, ln_b, out_g, out_b
    )
    nc = get_nc(cap, cap0)
    res = run_bass_kernel_spmd(nc, in_maps, core_ids=list(range(NCORES)))
    return unshard_output(
        res.results, core_tokens, zero_splits, input_tensor.shape,
        np.asarray(input_tensor).dtype,
    )



# revision 6
# speedup vs baseline: 2.1038x; 2.1038x over previous
"""Trainium2 Bass kernel for nn_MBSFeedForward (moe_routing) — fp8 edition.

Reference semantics (per token t with class c = type_seq[t]):
  c == 0:  out = LN_out(x_t)
  c >= 1:  e = c-1 (expert)
           u = GELU(x_t @ W1_e + b1_e) @ W2_e + b2_e
           v = LN_e(u + x_t)          (per-expert ln_g/ln_b)
           out = LN_out(v + x_t)      (out_g/out_b)

Sharding (host-side routing): 4 experts x 2 cores each; class-0 tokens
split over all 8 cores (outer LN only). No collectives; host scatters.

Device kernel design:
  - Both GEMMs run as fp8(e4m3) DoubleRow matmuls: each instruction
    contracts TWO 128-deep k-tiles (operands laid out [128, 2, n]) at
    0.5 PE cycles per output row. Host pre-quantizes x/W1/W2 with
    power-of-2 scales (RNE via ml_dtypes); the only on-device fp8
    rounding is the gelu->fp8 write, which the scalar engine does RNE
    (verified on HW). End-to-end rel err ~1.6e-2 vs the 2e-2 gate.
  - GEMM2 result arrives in PSUM scaled by sw2; LN is scale-invariant,
    so the residual is added as sw2*(x+b2) (host-precomputed, bf16) and
    the normalize needs no unscale. b2 folds into that residual.
  - LayerNorm: bn_stats/bn_aggr on DVE, rsqrt via bit-trick + 1 Newton
    step on GPSIMD (keeps Sqrt off the scalar engine so the Gelu
    activation table is loaded exactly once), normalize tensor_scalar
    on DVE. Residual adds on GPSIMD. LN intermediates in bf16, final
    normalize writes f32.
  - All inputs are DMA'd to SBUF once up front (everything fits);
    steady state does only output-store DMAs.
"""

import math
from contextlib import ExitStack

import numpy as np
import ml_dtypes

import concourse.bass as bass
import concourse.tile as tile
from concourse import bacc
from concourse import mybir
from concourse.bass_utils import run_bass_kernel_spmd

F32 = mybir.dt.float32
BF16 = mybir.dt.bfloat16
FP8 = mybir.dt.float8e4
I32 = mybir.dt.int32
E4M3 = ml_dtypes.float8_e4m3
NP_BF16 = ml_dtypes.bfloat16

P = 128
H = 768
F = 3072
KH = H // P     # 6  (k-tiles for GEMM1)
KF = F // P     # 24 (k-tiles for GEMM2)
TCH = 256       # tokens per chunk
TPT = TCH // P  # token tiles per chunk (2)
QK = 4          # kf-tiles per ps1/gelu batch
NQ = KF // QK   # 6 ps1 batches per chunk
NCORES = 8
RSQRT_MAGIC = 0x5F3759DF
FP8_TARGET = 192.0  # max|v*scale| target; e4m3 (IEEE) max finite is 240

ACT_FUNC = mybir.ActivationFunctionType.Gelu


def _pow2_scale(m: float) -> float:
    if m <= 0 or not math.isfinite(m):
        return 1.0
    return 2.0 ** math.floor(math.log2(FP8_TARGET / m))


def build_nc(cap: int, cap0: int, repeat: int = 1, *, b1_zero: bool = True,
             ln_trivial: bool = True, out_trivial: bool = True) -> bass.Bass:
    """Per-core Bass module for `cap` routed tokens and `cap0` LN-only
    tokens. repeat>1 re-runs the body (slope timing)."""
    assert cap % TCH == 0 and cap0 % P == 0
    nc = bacc.Bacc()
    chunks = cap // TCH
    nt0 = cap0 // P

    d_xrT = nc.dram_tensor("xrT", [H, cap], FP8, kind="ExternalInput")
    d_w1 = nc.dram_tensor("w1", [H, F], FP8, kind="ExternalInput")
    d_w2 = nc.dram_tensor("w2", [F, H], FP8, kind="ExternalInput")
    d_xr1 = nc.dram_tensor("xr1", [cap, H], BF16, kind="ExternalInput")
    d_xr0 = nc.dram_tensor("xr0", [cap, H], BF16, kind="ExternalInput")
    d_x0 = nc.dram_tensor("x0", [cap0, H], F32, kind="ExternalInput")
    d_ginv1 = nc.dram_tensor("ginv1", [1], F32, kind="ExternalInput")
    d_b1 = None if b1_zero else nc.dram_tensor("b1", [F], F32, kind="ExternalInput")
    d_lng = d_lnb = d_outg = d_outb = None
    if not ln_trivial:
        d_lng = nc.dram_tensor("lng", [H], F32, kind="ExternalInput")
        d_lnb = nc.dram_tensor("lnb", [H], F32, kind="ExternalInput")
    if not out_trivial:
        d_outg = nc.dram_tensor("outg", [H], F32, kind="ExternalInput")
        d_outb = nc.dram_tensor("outb", [H], F32, kind="ExternalInput")
    d_yr = nc.dram_tensor("yr", [cap, H], F32, kind="ExternalOutput")
    d_y0 = nc.dram_tensor("y0", [cap0, H], F32, kind="ExternalOutput")

    with tile.TileContext(nc) as tc, ExitStack() as ctx:
        singles = ctx.enter_context(tc.tile_pool(name="singles", bufs=1))
        hpool = ctx.enter_context(tc.tile_pool(name="hact", bufs=8))
        wbf = ctx.enter_context(tc.tile_pool(name="wbf", bufs=6))
        wf32 = ctx.enter_context(tc.tile_pool(name="wf32", bufs=4))
        stat = ctx.enter_context(tc.tile_pool(name="stat", bufs=24))
        ps1p = ctx.enter_context(tc.tile_pool(name="ps1", bufs=2, space="PSUM"))
        psUp = ctx.enter_context(tc.tile_pool(name="psU", bufs=2, space="PSUM"))

        # --- resident inputs, sliced so early consumers start early ---
        w1sb = singles.tile([P, KH, F], FP8)
        w1_v = d_w1[:].rearrange("(ko ki) f -> ki ko f", ki=P)
        w2sb = singles.tile([P, KF, H], FP8)
        w2_v = d_w2[:].rearrange("(ko ki) h -> ki ko h", ki=P)
        xrTsb = singles.tile([P, KH, cap], FP8)
        xrT_v = d_xrT[:].rearrange("(ko ki) t -> ki ko t", ki=P)
        # chunk 0's operands first: w1 q0, xrT c0, then w2 pairs for c0,
        # then the rest
        nc.gpsimd.dma_start(w1sb[:, :, 0:QK * P], w1_v[:, :, 0:QK * P])
        nc.gpsimd.dma_start(xrTsb[:, :, 0:TCH], xrT_v[:, :, 0:TCH])
        for q in range(1, NQ):
            nc.gpsimd.dma_start(
                w1sb[:, :, q * QK * P:(q + 1) * QK * P],
                w1_v[:, :, q * QK * P:(q + 1) * QK * P])
        for pr in range(KF // 2):
            nc.gpsimd.dma_start(w2sb[:, 2 * pr:2 * pr + 2], w2_v[:, 2 * pr:2 * pr + 2])
        for c in range(1, chunks):
            nc.gpsimd.dma_start(xrTsb[:, :, c * TCH:(c + 1) * TCH],
                                xrT_v[:, :, c * TCH:(c + 1) * TCH])

        ntile = cap // P
        xr1sb = singles.tile([P, ntile, H], BF16)
        xr1_v = d_xr1[:].rearrange("(n p) h -> p n h", p=P)
        xr0sb = singles.tile([P, ntile, H], BF16)
        xr0_v = d_xr0[:].rearrange("(n p) h -> p n h", p=P)
        for n in range(ntile):
            nc.sync.dma_start(xr1sb[:, n], xr1_v[:, n])
            nc.sync.dma_start(xr0sb[:, n], xr0_v[:, n])
        x0sb = singles.tile([P, max(nt0, 1), H], F32)
        x0_v = d_x0[:].rearrange("(n p) h -> p n h", p=P)
        for n in range(nt0):
            nc.sync.dma_start(x0sb[:, n], x0_v[:, n])

        ginv1 = singles.tile([P, 1], F32, name="ginv1")
        nc.gpsimd.dma_start(ginv1, d_ginv1[:][None, :].to_broadcast([P, 1]))

        def bc_tile(d, nm):
            t = singles.tile([P, H], F32, tag=nm, name=nm)
            nc.gpsimd.dma_start(t, d[:][None, :].to_broadcast([P, H]))
            return t

        lngbc = lnbbc = outgbc = outbbc = None
        if not ln_trivial:
            lngbc, lnbbc = bc_tile(d_lng, "lngbc"), bc_tile(d_lnb, "lnbbc")
        if not out_trivial:
            outgbc, outbbc = bc_tile(d_outg, "outgbc"), bc_tile(d_outb, "outbbc")
        b1sb = None
        if not b1_zero:
            b1sb = singles.tile([P, KF], F32, name="b1sb")
            nc.gpsimd.dma_start(b1sb, d_b1[:].rearrange("(o p) -> p o", p=P))

        def layernorm(src, dst, gbc, bbc):
            """dst = LN(src) (normalize; optional affine). src/dst [P, H].
            Stats+normalize on DVE; rsqrt bit-trick + Newton on GPSIMD."""
            st = stat.tile([P, 2, 6], F32, tag="st")
            nc.vector.bn_stats(st[:, 0], src[:, 0:384])
            nc.vector.bn_stats(st[:, 1], src[:, 384:768])
            mv = stat.tile([P, 2], F32, tag="mv")
            nc.vector.bn_aggr(mv, st)
            v = mv[:, 1:2]
            # quake seed y0 = float_bits(magic - (v_int >> 1)); there is no
            # reversed subtract, so compute it as (~(v>>1)) + (magic+1).
            # shift+xor fuse (both bitwise); the int add is a second op.
            # GPSIMD fails the ISA check for these — they run on DVE.
            yi = stat.tile([P, 1], I32, tag="yi")
            nc.vector.tensor_scalar(
                yi, v.bitcast(I32), scalar1=1, scalar2=-1,
                op0=mybir.AluOpType.logical_shift_right,
                op1=mybir.AluOpType.bitwise_xor)
            nc.vector.tensor_scalar(
                yi, yi, scalar1=RSQRT_MAGIC + 1, scalar2=None,
                op0=mybir.AluOpType.add)
            y0 = yi.bitcast(F32)
            # one Newton step: rs = y0*(1.5 - 0.5*v*y0^2), with the inner
            # tensor_scalar computing (-0.5*z) - (-1.5) to keep the sign
            z = stat.tile([P, 1], F32, tag="z")
            nc.gpsimd.tensor_tensor(z, y0, y0, op=mybir.AluOpType.mult)
            nc.gpsimd.tensor_tensor(z, z, v, op=mybir.AluOpType.mult)
            nc.gpsimd.tensor_scalar(
                z, z, scalar1=-0.5, scalar2=-1.5,
                op0=mybir.AluOpType.mult, op1=mybir.AluOpType.subtract)
            rs = stat.tile([P, 1], F32, tag="rs")
            nc.gpsimd.tensor_tensor(rs, z, y0, op=mybir.AluOpType.mult)
            nc.vector.tensor_scalar(
                dst, src, scalar1=mv[:, 0:1], scalar2=rs,
                op0=mybir.AluOpType.subtract, op1=mybir.AluOpType.mult)
            if gbc is not None:
                nc.gpsimd.tensor_mul(dst, dst, gbc)
                nc.gpsimd.tensor_add(dst, dst, bbc)

        def x0_tile(n):
            o = wf32.tile([P, H], F32, tag="o")
            layernorm(x0sb[:, n], o, outgbc, outbbc)
            nc.sync.dma_start(d_y0[n * P:(n + 1) * P], o)

        # x0 positions: spread LN-only tiles across the chunk loop
        x0_at = {}
        if nt0:
            for i in range(nt0):
                x0_at.setdefault(min(i * max(1, chunks // nt0), chunks - 1), []).append(i)

        for it in range(repeat * chunks):
            c = it % chunks
            # --- GEMM1 + gelu: 6 batches of 4 kf-tiles ---
            has = []
            for q in range(NQ):
                ps1 = ps1p.tile([P, QK, TCH], F32, tag="ps1")
                for j in range(QK):
                    kf = q * QK + j
                    for t in range(KH // 2):
                        nc.tensor.matmul(
                            ps1[:, j],
                            w1sb[:, 2 * t:2 * t + 2, kf * P:(kf + 1) * P],
                            xrTsb[:, 2 * t:2 * t + 2, c * TCH:(c + 1) * TCH],
                            start=(t == 0), stop=(t == KH // 2 - 1),
                            perf_mode=mybir.MatmulPerfMode.DoubleRow)
                ha = hpool.tile([P, QK, TCH], FP8, tag="ha")
                if b1_zero:
                    nc.scalar.activation(ha, ps1, ACT_FUNC, bias=0.0, scale=ginv1)
                else:
                    for j in range(QK):
                        kf = q * QK + j
                        nc.scalar.activation(
                            ha[:, j], ps1[:, j], ACT_FUNC,
                            bias=b1sb[:, kf:kf + 1], scale=ginv1)
                has.append(ha)
            # --- GEMM2 + LN chain per 128-token tile ---
            for m in range(TPT):
                psU = psUp.tile([P, H], F32, tag="psU")
                for pr in range(KF // 2):
                    q, b = divmod(pr, QK // 2)
                    lhsT = has[q][:, 2 * b:2 * b + 2, m * P:(m + 1) * P]
                    for h0, h1 in ((0, 512), (512, H)):
                        nc.tensor.matmul(
                            psU[:, h0:h1], lhsT,
                            w2sb[:, 2 * pr:2 * pr + 2, h0:h1],
                            start=(pr == 0), stop=(pr == KF // 2 - 1),
                            perf_mode=mybir.MatmulPerfMode.DoubleRow)
                n = c * TPT + m
                t1 = wbf.tile([P, H], BF16, tag="t1")
                # GPSIMD can't read PSUM — this add stays on DVE
                nc.vector.tensor_tensor(t1, psU, xr1sb[:, n], op=mybir.AluOpType.add)
                v1 = wbf.tile([P, H], BF16, tag="v1")
                layernorm(t1, v1, lngbc, lnbbc)
                t2 = wbf.tile([P, H], BF16, tag="t2")
                nc.gpsimd.tensor_tensor(t2, v1, xr0sb[:, n], op=mybir.AluOpType.add)
                o = wf32.tile([P, H], F32, tag="o")
                layernorm(t2, o, outgbc, outbbc)
                nc.sync.dma_start(d_yr[n * P:(n + 1) * P], o)
            for i in x0_at.get(c, ()):
                x0_tile((it // chunks) * 0 + i)

    nc.finalize()
    return nc


_NC_CACHE: dict[tuple, bass.Bass] = {}


def get_nc(cap: int, cap0: int, repeat: int = 1,
           flags: tuple = (True, True, True)) -> bass.Bass:
    key = (cap, cap0, repeat, flags)
    if key not in _NC_CACHE:
        b1z, lnt, outt = flags
        _NC_CACHE[key] = build_nc(cap, cap0, repeat, b1_zero=b1z,
                                  ln_trivial=lnt, out_trivial=outt)
    return _NC_CACHE[key]


def _round_up(n: int, m: int) -> int:
    return max(m, ((n + m - 1) // m) * m)


def shard_inputs(input_tensor, type_seq, W1, b1, W2, b2, ln_g, ln_b, out_g, out_b):
    """Host-side routing + fp8/bf16 prep. Returns (in_maps, core_tokens,
    zero_splits, cap, cap0, flags)."""
    B, L, _H = input_tensor.shape
    assert _H == H, f"kernel hardcodes d_model={H}, got {_H}"
    x = np.ascontiguousarray(np.asarray(input_tensor, dtype=np.float32)).reshape(B * L, H)
    ts_flat = np.asarray(type_seq).reshape(-1).astype(np.int64)
    NB = W1.shape[0]
    per_expert = max(1, NCORES // NB)
    W1 = np.asarray(W1, dtype=np.float32)
    W2 = np.asarray(W2, dtype=np.float32)
    b1 = np.asarray(b1, dtype=np.float32)
    b2 = np.asarray(b2, dtype=np.float32)

    flags = (
        not b1.any(),
        bool(np.all(ln_g == 1.0) and not np.asarray(ln_b).any()),
        bool(np.all(out_g == 1.0) and not np.asarray(out_b).any()),
    )

    core_tokens = []
    core_expert = []
    for e in range(NB):
        toks = np.nonzero(ts_flat == e + 1)[0]
        for s in np.array_split(toks, per_expert):
            core_tokens.append(s)
            core_expert.append(e)
    while len(core_tokens) < NCORES:
        core_tokens.append(np.zeros(0, dtype=np.int64))
        core_expert.append(0)
    zero_splits = np.array_split(np.nonzero(ts_flat == 0)[0], NCORES)

    cap = _round_up(max(len(t) for t in core_tokens), TCH)
    cap0 = _round_up(max(len(z) for z in zero_splits), P)

    sx = _pow2_scale(float(np.abs(x).max()))
    xq = (x * np.float32(sx)).astype(E4M3)  # global; sliced per core

    sw1 = [_pow2_scale(float(np.abs(W1[e]).max())) for e in range(NB)]
    sw2 = [_pow2_scale(float(np.abs(W2[e]).max())) for e in range(NB)]
    w1q = [(W1[e] * np.float32(sw1[e])).astype(E4M3) for e in range(NB)]
    w2q = [(W2[e] * np.float32(sw2[e])).astype(E4M3) for e in range(NB)]

    def f32c(a):
        return np.ascontiguousarray(np.asarray(a, dtype=np.float32))

    in_maps = []
    for cidx in range(NCORES):
        toks = core_tokens[cidx]
        e = core_expert[cidx]
        z = zero_splits[cidx]
        xrT = np.zeros((H, cap), E4M3)
        xrT[:, : len(toks)] = xq[toks].T
        xr1 = np.zeros((cap, H), NP_BF16)
        xr1[: len(toks)] = ((x[toks] + b2[e]) * np.float32(sw2[e])).astype(NP_BF16)
        xr0 = np.zeros((cap, H), NP_BF16)
        xr0[: len(toks)] = x[toks].astype(NP_BF16)
        x0 = np.zeros((cap0, H), np.float32)
        x0[: len(z)] = x[z]
        im = {
            "xrT": np.ascontiguousarray(xrT),
            "w1": w1q[e],
            "w2": w2q[e],
            "xr1": xr1,
            "xr0": xr0,
            "x0": x0,
            "ginv1": np.array([1.0 / (sx * sw1[e])], np.float32),
        }
        if not flags[0]:
            im["b1"] = f32c(b1[e])
        if not flags[1]:
            im["lng"] = f32c(ln_g[e])
            im["lnb"] = f32c(ln_b[e])
        if not flags[2]:
            im["outg"] = f32c(out_g)
            im["outb"] = f32c(out_b)
        in_maps.append(im)
    return in_maps, core_tokens, zero_splits, cap, cap0, flags


def unshard_output(results, core_tokens, zero_splits, shape, dtype):
    B, L, _H = shape
    out = np.empty((B * L, H), np.float32)
    for c in range(NCORES):
        toks = core_tokens[c]
        z = zero_splits[c]
        if len(toks):
            out[toks] = results[c]["yr"][: len(toks)]
        if len(z):
            out[z] = results[c]["y0"][: len(z)]
    return out.reshape(B, L, H).astype(dtype, copy=False)


def kernel(input_tensor, type_seq, W1, b1, W2, b2, ln_g, ln_b, out_g, out_b):
    in_maps, core_tokens, zero_splits, cap, cap0, flags = shard_inputs(
        input_tensor, type_seq, W1, b1, W2, b2, ln_g, ln_b, out_g, out_b
    )
    nc = get_nc(cap, cap0, flags=flags)
    res = run_bass_kernel_spmd(nc, in_maps, core_ids=list(range(NCORES)))
    return unshard_output(
        res.results, core_tokens, zero_splits, input_tensor.shape,
        np.asarray(input_tensor).dtype,
    )
